# revision 1
# baseline (speedup 1.0000x reference)
"""Trainium2 Bass kernel for nn_Cross_Attention_18425409700231.

Per-sample channel attention (16 heads x 8 channels, L2-normalized over
spatial, softmax over in-head channels) followed by a conv block
(3x3 conv -> LeakyReLU -> 1x1 conv -> reflect-pad depthwise 3x3 ->
LeakyReLU, plus 1x1 shortcut) and a residual add.

Sharding: data-parallel over batch B=8 -> one sample per NeuronCore.

Device algorithm per core (sample b), all layouts [C=128 partitions, H*W]:
  A. Gram matrix G = x1 @ x1^T (contract over 16384 spatial) via
     PE-transposed bf16 chunks; norms from diag(G); S = rn*G*rn (one PE
     transpose for the column scale, exploiting symmetry); E = exp(S*temp)
     masked to the 16 block-diagonal 8x8 head blocks.
  B. Attention apply: P = E @ x2 (f32r matmuls), row-scaled by 1/rowsum(E)
     during the PSUM->SBUF copy, written into a zero-padded 130x130 buffer.
  C. conv1 3x3 as 9 accumulated matmuls per 4-row band from the padded
     buffer; bias+LeakyReLU fused in the PSUM->SBUF copy into a second
     (reflect-)padded buffer.
  D. conv2(1x1) and depthwise 3x3 fused into 9 taps of modified weights
     W2t[t] = dw_w[:,t] * conv2_w (reflect pad commutes with 1x1 conv);
     bias'+LeakyReLU; shortcut 1x1 matmul on the attention output; final
     out = lrelu(...) + (sc + sc_b) + x1, streamed back to DRAM.
  B/C/D are emitted interleaved per 4-row band (with dependency lags) so
  the PE never drains while DMA streams x2/x1 in and the result out.
"""

import numpy as np
import ml_dtypes

B, C, H, W = 8, 128, 128, 128
HW = H * W
HEADS, HEAD_C = 16, 8
SLOPE = 0.2
EPS = 1e-12
PW = W + 2  # padded width
NB = H // 4  # 32 bands of 4 rows

_cache = {}


def _build_program(debug=False):
    import concourse.bass as bass
    import concourse.tile as tile
    import concourse.mybir as mybir
    from concourse import bacc

    dt = mybir.dt
    f32, f32r, bf16 = dt.float32, dt.float32r, dt.bfloat16
    u32 = dt.uint32
    AF = mybir.ActivationFunctionType
    ALU = mybir.AluOpType
    AX = mybir.AxisListType

    nc = bacc.Bacc("TRN2", num_devices=8)

    x1 = nc.dram_tensor("x1", [C, HW], f32, kind="ExternalInput").ap()
    x1h = nc.dram_tensor("x1h", [C, HW], bf16, kind="ExternalInput").ap()
    x2 = nc.dram_tensor("x2", [C, HW], f32r, kind="ExternalInput").ap()
    wc1 = nc.dram_tensor("wc1", [C, 9, C], f32r, kind="ExternalInput").ap()
    wc2 = nc.dram_tensor("wc2", [C, 9, C], f32r, kind="ExternalInput").ap()
    wsc = nc.dram_tensor("wsc", [C, C], f32r, kind="ExternalInput").ap()
    scl = nc.dram_tensor("scl", [C, 4], f32, kind="ExternalInput").ap()
    bmask = nc.dram_tensor("bmask", [C, C], f32, kind="ExternalInput").ap()
    idf = nc.dram_tensor("idf", [C, C], f32, kind="ExternalInput").ap()
    idb = nc.dram_tensor("idb", [C, C], bf16, kind="ExternalInput").ap()
    out = nc.dram_tensor("out", [C, HW], f32, kind="ExternalOutput").ap()
    if debug:
        dbg_g = nc.dram_tensor("dbg_g", [C, C], f32, kind="ExternalOutput").ap()
        dbg_em = nc.dram_tensor("dbg_em", [C, C], f32, kind="ExternalOutput").ap()
        dbg_rinv = nc.dram_tensor("dbg_rinv", [C, 1], f32, kind="ExternalOutput").ap()
        dbg_ph = nc.dram_tensor("dbg_ph", [C, (H + 2) * PW], f32, kind="ExternalOutput").ap()

    taps = [(dy, dx) for dy in range(3) for dx in range(3)]

    with tile.TileContext(nc) as tc:
        with (
            tc.tile_pool(name="consts", bufs=1) as consts,
            tc.tile_pool(name="pads", bufs=1) as pads,
            tc.tile_pool(name="attn", bufs=1) as attn,
            tc.tile_pool(name="streams", bufs=2) as streams,
            tc.tile_pool(name="bands", bufs=3) as bands,
        ):
            # ---- constants to SBUF ----
            idbs = consts.tile([C, C], bf16)
            nc.sync.dma_start(out=idbs, in_=idb)
            w1s = consts.tile([C, 9, C], f32r)
            nc.gpsimd.dma_start(out=w1s, in_=wc1)
            w2s = consts.tile([C, 9, C], f32r)
            nc.gpsimd.dma_start(out=w2s, in_=wc2)
            wscs = consts.tile([C, C], f32r)
            nc.gpsimd.dma_start(out=wscs, in_=wsc)
            scls = consts.tile([C, 4], f32)
            nc.gpsimd.dma_start(out=scls, in_=scl)
            bmasks = consts.tile([C, C], f32)
            nc.gpsimd.dma_start(out=bmasks, in_=bmask)
            idfs = consts.tile([C, C], f32)
            nc.gpsimd.dma_start(out=idfs, in_=idf)
            b1_ap = scls[:, 0:1]
            b2_ap = scls[:, 1:2]
            bsc_ap = scls[:, 2:3]
            temp_ap = scls[:, 3:4]

            # ---- persistent padded buffers ----
            p2x = pads.tile([C, H + 2, PW], f32r)  # x2, zero-pad
            ph = pads.tile([C, H + 2, PW], f32r)   # conv1 out, reflect-pad

            # zero borders of p2x; interior streamed straight from DRAM
            nc.vector.memset(p2x[:, 0:1, :].bitcast(u32), 0)
            nc.vector.memset(p2x[:, H + 1 : H + 2, :].bitcast(u32), 0)
            nc.vector.memset(p2x[:, 1 : H + 1, 0:1].bitcast(u32), 0)
            nc.vector.memset(p2x[:, 1 : H + 1, PW - 1 : PW].bitcast(u32), 0)

            # ================= phase A: Gram + softmax =================
            with (
                tc.tile_pool(name="psG", bufs=1, space="PSUM") as psG,
                tc.tile_pool(name="psT", bufs=3, space="PSUM") as psT,
            ):
                gps = psG.tile([C, C], f32)
                kk = 0
                col0 = 0
                for ncols in (1024, 3072, 4096, 4096, 4096):
                    x1bt = streams.tile(
                        [C, ncols], bf16, bufs=3, tag="x1bt", name="x1bt"
                    )
                    nc.sync.dma_start(
                        out=x1bt, in_=x1h[:, col0 : col0 + ncols]
                    )
                    col0 += ncols
                    for g in range(ncols // 512):  # batches of 4 chunks of 128
                        tp = psT.tile([C, 4, C], bf16)
                        for i in range(4):
                            k = g * 4 + i
                            nc.tensor.transpose(
                                out=tp[:, i, :],
                                in_=x1bt[:, k * 128 : (k + 1) * 128],
                                identity=idbs,
                            )
                        tsb = streams.tile([C, 4, C], bf16, bufs=4)
                        nc.vector.tensor_copy(out=tsb, in_=tp)
                        for i in range(4):
                            nc.tensor.matmul(
                                out=gps,
                                lhsT=tsb[:, i, :],
                                rhs=tsb[:, i, :],
                                start=(kk == 0),
                                stop=(kk == 127),
                                skip_group_check=True,
                            )
                            kk += 1

                # stream x2 into the padded buffer (no deps; overlaps A tail)
                for j in range(8):
                    nc.sync.dma_start(
                        out=p2x[:, 1 + 16 * j : 17 + 16 * j, 1 : 1 + C],
                        in_=x2[:, j * 2048 : (j + 1) * 2048],
                    )

                # diag -> norms -> rn
                gi = attn.tile([C, C], f32)
                nc.vector.tensor_mul(out=gi, in0=gps, in1=idfs)
                diag = attn.tile([C, 1], f32)
                nc.vector.reduce_sum(out=diag, in_=gi, axis=AX.X)
                norm = attn.tile([C, 1], f32)
                nc.scalar.activation(out=norm, in_=diag, func=AF.Sqrt)
                nc.vector.tensor_scalar_max(out=norm, in0=norm, scalar1=EPS)
                rn = attn.tile([C, 1], f32)
                nc.vector.reciprocal(out=rn, in_=norm)

                # S = diag(rn) G diag(rn) via row-scale, transpose, row-scale
                s1 = attn.tile([C, C], f32)
                nc.vector.tensor_scalar_mul(out=s1, in0=gps, scalar1=rn)
                with tc.tile_pool(name="psS", bufs=1, space="PSUM") as psS:
                    s1t = psS.tile([C, C], f32)
                    nc.tensor.transpose(out=s1t, in_=s1, identity=idfs)
                    s2 = attn.tile([C, C], f32)
                    nc.vector.tensor_scalar_mul(out=s2, in0=s1t, scalar1=rn)

                # E = exp(S * temp) * blockmask ; rinv = 1/rowsum(E)
                e0 = attn.tile([C, C], f32)
                nc.scalar.activation(out=e0, in_=s2, func=AF.Exp, scale=temp_ap)
                em = attn.tile([C, C], f32r)
                nc.vector.tensor_mul(out=em, in0=e0, in1=bmasks)
                rs = attn.tile([C, 1], f32)
                nc.vector.reduce_sum(out=rs, in_=em, axis=AX.X)
                rinv = attn.tile([C, 1], f32)
                nc.vector.reciprocal(out=rinv, in_=rs)

                # fused attention+conv weights: L_t = E diag(rinv) w1s_t,
                # Lsc = E diag(rinv) wsc  (E symmetric), so that
                # conv1(P) = sum_t L_t^T @ x2_shift with P never materialized.
                ltp = []
                with tc.tile_pool(name="psW", bufs=2, space="PSUM") as psW:
                    for p in range(5):  # pairs of taps -> N=256 matmuls
                        rt = attn.tile([C, 2, C], f32r, name=f"rt{p}", tag="rt")
                        if p < 4:
                            nc.vector.tensor_scalar_mul(
                                out=rt, in0=w1s[:, 2 * p : 2 * p + 2, :], scalar1=rinv
                            )
                        else:
                            nc.vector.tensor_scalar_mul(
                                out=rt[:, 0, :], in0=w1s[:, 8, :], scalar1=rinv
                            )
                            nc.vector.tensor_scalar_mul(
                                out=rt[:, 1, :], in0=wscs, scalar1=rinv
                            )
                        lps = psW.tile([C, 2, C], f32, name=f"lps{p}", tag="lps")
                        nc.tensor.matmul(
                            out=lps, lhsT=em, rhs=rt, start=True, stop=True
                        )
                        lt = attn.tile([C, 2, C], f32r, name=f"lt{p}")
                        nc.scalar.activation(out=lt, in_=lps, func=AF.Copy)
                        ltp.append(lt)
                lts = [ltp[t // 2][:, t % 2, :] for t in range(10)]
                if debug:
                    gsb = attn.tile([C, C], f32)
                    nc.vector.tensor_copy(out=gsb, in_=gps)
                    nc.sync.dma_start(out=dbg_g, in_=gsb)
                    nc.gpsimd.dma_start(out=dbg_em, in_=em)
                    nc.sync.dma_start(out=dbg_rinv, in_=rinv)

            # ============ phases C/D interleaved per 4-row band ============
            # C band b reads p2x rows 4b-1..4b+4 (streamed-in x2)
            # D band b reads ph  rows 4b-1..4b+4  -> needs C bands <= b+1
            with (
                tc.tile_pool(name="psC", bufs=3, space="PSUM") as psC,
                tc.tile_pool(name="psD", bufs=3, space="PSUM") as psD,
                tc.tile_pool(name="psS2", bufs=2, space="PSUM") as psS2,
                tc.tile_pool(name="x1st", bufs=2) as x1st,
            ):
                state = {}

                def emit_C(b):
                    y0 = 4 * b
                    cps = psC.tile([C, 4, C], f32)
                    for t, (ddy, ddx) in enumerate(taps):
                        nc.tensor.matmul(
                            out=cps,
                            lhsT=lts[t],
                            rhs=p2x[:, y0 + ddy : y0 + ddy + 4, ddx : ddx + C],
                            start=(t == 0),
                            stop=(t == 8),
                        )
                    nc.scalar.activation(
                        out=ph[:, 1 + y0 : 5 + y0, 1 : 1 + C],
                        in_=cps,
                        func=AF.Prelu,
                        bias=b1_ap,
                        alpha=SLOPE,
                    )
                    # incremental reflect pad of the left/right columns
                    nc.gpsimd.tensor_copy(
                        out=ph[:, 1 + y0 : 5 + y0, 0:1],
                        in_=ph[:, 1 + y0 : 5 + y0, 2:3],
                    )
                    nc.gpsimd.tensor_copy(
                        out=ph[:, 1 + y0 : 5 + y0, PW - 1 : PW],
                        in_=ph[:, 1 + y0 : 5 + y0, PW - 3 : PW - 2],
                    )
                    if b == 0:
                        # reflect pad top row (incl. corners)
                        nc.gpsimd.tensor_copy(out=ph[:, 0:1, :], in_=ph[:, 2:3, :])
                    if b == NB - 1:
                        nc.gpsimd.tensor_copy(
                            out=ph[:, H + 1 : H + 2, :], in_=ph[:, H - 1 : H, :]
                        )

                def emit_D(b):
                    y0 = 4 * b
                    if b % 2 == 0:
                        x1b2 = x1st.tile([C, 1024], f32)
                        nc.gpsimd.dma_start(
                            out=x1b2, in_=x1[:, y0 * W : (y0 + 8) * W]
                        )
                        otile = x1st.tile([C, 1024], f32, tag="otile")
                        state["x1b2"] = x1b2
                        state["otile"] = otile
                    x1b2 = state["x1b2"]
                    otile = state["otile"]
                    x1b = x1b2[:, (b % 2) * 512 : (b % 2 + 1) * 512].rearrange(
                        "p (a b) -> p a b", a=4
                    )
                    dps = psD.tile([C, 4, C], f32)
                    for t, (ddy, ddx) in enumerate(taps):
                        nc.tensor.matmul(
                            out=dps,
                            lhsT=w2s[:, t, :],
                            rhs=ph[:, y0 + ddy : y0 + ddy + 4, ddx : ddx + C],
                            start=(t == 0),
                            stop=(t == 8),
                        )
                    sps = psS2.tile([C, 4, C], f32)
                    nc.tensor.matmul(
                        out=sps,
                        lhsT=lts[9],
                        rhs=p2x[:, 1 + y0 : 5 + y0, 1 : 1 + C],
                        start=True,
                        stop=True,
                    )
                    h3 = bands.tile([C, 4, C], f32)
                    nc.scalar.activation(
                        out=h3, in_=dps, func=AF.Prelu, bias=b2_ap, alpha=SLOPE
                    )
                    ob = otile[:, (b % 2) * 512 : (b % 2 + 1) * 512].rearrange(
                        "p (a b) -> p a b", a=4
                    )
                    # (sc + bsc) + x1 runs on DVE in parallel with the
                    # Prelu on ACT; h3 joins last.
                    nc.vector.scalar_tensor_tensor(
                        out=ob,
                        in0=sps,
                        scalar=bsc_ap,
                        in1=x1b,
                        op0=ALU.add,
                        op1=ALU.add,
                    )
                    nc.vector.tensor_add(out=ob, in0=ob, in1=h3)
                    if b == NB - 2:
                        # flush first half of the last pair early
                        nc.sync.dma_start(
                            out=out[:, y0 * W : (y0 + 4) * W], in_=otile[:, 0:512]
                        )
                    elif b == NB - 1:
                        nc.sync.dma_start(
                            out=out[:, y0 * W : (y0 + 4) * W], in_=otile[:, 512:1024]
                        )
                    elif b % 2 == 1:
                        nc.sync.dma_start(
                            out=out[:, (y0 - 4) * W : (y0 + 4) * W], in_=otile
                        )

                for k in range(NB + 1):
                    if k < NB:
                        emit_C(k)
                    if k >= 1:
                        emit_D(k - 1)
                        if debug and k == NB:
                            nc.gpsimd.dma_start(out=dbg_ph, in_=ph)

    nc.compile()
    return nc


def _prep_consts(temperature, conv1_w, conv2_w, dw_w, conv1_b, conv2_b, dw_b, sc_b, sc_w):
    f32 = np.float32
    conv1_w = np.asarray(conv1_w, f32)
    conv2_w = np.asarray(conv2_w, f32)
    dw_w = np.asarray(dw_w, f32)
    sc_w = np.asarray(sc_w, f32)
    # conv1 taps as lhsT: wc1[ci, t, co] = conv1_w[co, ci, dy, dx]
    wc1 = np.ascontiguousarray(conv1_w.transpose(1, 2, 3, 0).reshape(C, 9, C))
    # fused conv2+dw taps: wc2[ci, t, co] = conv2_w[co, ci] * dw_w[co, t]
    A2 = conv2_w[:, :, 0, 0]                      # [co, ci]
    Dw = dw_w[:, 0, :, :].reshape(C, 9)           # [co, t]
    wc2 = np.ascontiguousarray(np.einsum("oc,ot->cto", A2, Dw).astype(f32))
    wsc = np.ascontiguousarray(sc_w[:, :, 0, 0].T.astype(f32))
    b2p = np.asarray(dw_b, f32) + np.asarray(conv2_b, f32) * Dw.sum(axis=1)
    temp_b = np.repeat(np.asarray(temperature, f32).reshape(HEADS), HEAD_C)
    scl = np.ascontiguousarray(
        np.stack(
            [np.asarray(conv1_b, f32), b2p, np.asarray(sc_b, f32), temp_b], axis=1
        )
    )  # [128, 4]
    bmask = np.kron(np.eye(HEADS, dtype=f32), np.ones((HEAD_C, HEAD_C), f32))
    idf = np.eye(C, dtype=f32)
    idb = np.eye(C, dtype=ml_dtypes.bfloat16)
    return dict(
        wc1=wc1, wc2=wc2, wsc=wsc, scl=scl,
        bmask=np.ascontiguousarray(bmask),
        idf=np.ascontiguousarray(idf),
        idb=np.ascontiguousarray(idb),
    )


def kernel(
    x1, x2, temperature, conv1_w, conv1_b, conv2_w, conv2_b, dw_w, dw_b, sc_w, sc_b
):
    from concourse.bass_utils import run_bass_kernel_spmd

    if "nc" not in _cache:
        _cache["nc"] = _build_program()
    nc = _cache["nc"]

    x1 = np.ascontiguousarray(np.asarray(x1, np.float32))
    x2 = np.ascontiguousarray(np.asarray(x2, np.float32))
    consts = _prep_consts(
        temperature, conv1_w, conv2_w, dw_w, conv1_b, conv2_b, dw_b, sc_b, sc_w
    )
    in_maps = []
    for b in range(B):
        m = dict(consts)
        m["x1"] = x1[b].reshape(C, HW)
        m["x1h"] = x1[b].reshape(C, HW).astype(ml_dtypes.bfloat16)
        m["x2"] = x2[b].reshape(C, HW)
        in_maps.append(m)

    res = run_bass_kernel_spmd(nc, in_maps, core_ids=list(range(B)))
    outs = [res.results[b]["out"].reshape(C, H, W) for b in range(B)]
    return np.stack(outs, axis=0)



# revision 3
# speedup vs baseline: 1.7013x; 1.7013x over previous
"""Trainium2 Bass kernel for nn_Cross_Attention_18425409700231.

Per-sample channel attention (16 heads x 8 channels, L2-normalized over
spatial, softmax over in-head channels) followed by a conv block
(3x3 conv -> LeakyReLU -> 1x1 conv -> reflect-pad depthwise 3x3 ->
LeakyReLU, plus 1x1 shortcut) and a residual add.

Sharding: data-parallel over batch B=8 -> one sample per NeuronCore.

Device algorithm per core (sample b), layouts [C=128 partitions, H*W]:
  A. Gram G = x1 @ x1^T from a host-pretransposed bf16 copy of x1
     (x1t[p, 128k+c] = x1[c, 128k+p]) -> 128 plain bf16 matmuls, no
     PE transposes and no PSUM->SBUF copies.
  B. diag(G) -> norms; S = rn*G*rn (one PE transpose); E = exp(S*temp)
     masked to the 16 block-diagonal 8x8 head blocks; rinv = 1/rowsum.
     Fused attention+conv weights L_t = E diag(rinv*64) w1_t (and the
     shortcut column), quantized to fp8-e4m3 on the fly; the shortcut
     weights are split hi/lo into two fp8 k-tiles for ~bf16 accuracy.
  C. conv1 3x3 over fp8 x2 (hosted-quantized at scale 16) as 5
     fp8 DoubleRow matmuls per 4-row band (taps paired into 2x128
     k-tiles via overlapping access patterns; the PE runs DoubleRow at
     0.5 cycles/row = 4x the f32r rate). Bias+LeakyReLU+rescale fused
     in the PSUM->SBUF copy, written as fp8 (scale 4) into a
     reflect-padded buffer.
  D. conv2(1x1)+depthwise 3x3 fused into 9 dense taps (reflect pad
     commutes with 1x1 conv), again 5 DoubleRow matmuls per band, plus
     one DoubleRow shortcut matmul (hi/lo k-tiles, stride-0 pair dim).
     LeakyReLU on DVE via max(x, 0.2x); final out = h3 + sc + x1
     (bf16 residual) assembled across DVE/Pool, stored bf16 and
     upcast to f32 on the host.
  C/D are emitted interleaved per 4-row band so the PE never drains.
"""

import numpy as np
import ml_dtypes
import bass_rust

B, C, H, W = 8, 128, 128, 128
HW = H * W
HEADS, HEAD_C = 16, 8
SLOPE = 0.2
EPS = 1e-12
PW = W + 2  # padded width
NB = H // 4  # 32 bands of 4 rows

# fp8 scaling factors
S_X2 = 16.0   # x2 quantized at e4m3(16*x2)
S_L = 64.0    # fused conv1/shortcut weights
S_H = 4.0     # h1 (conv1 output) fp8 scale
S_W2 = 256.0  # fused conv2+dw weights
# conv1 PSUM carries S_L*S_X2 = 1024x; conv2 PSUM carries S_W2*S_H = 1024x;
# shortcut PSUM carries S_L*S_X2 = 1024x.
PS_INV = 1.0 / 1024.0

E4 = ml_dtypes.float8_e4m3
BF16 = ml_dtypes.bfloat16

_cache = {}

# conv taps row-major and their pair layout: pairs (2p, 2p+1), the 5th
# pair is (tap8, zero-weights dummy).  delta = in-partition element offset
# from tap 2p to tap 2p+1 inside the [*, 130, 130] padded buffer.
TAPS = [(dy, dx) for dy in range(3) for dx in range(3)]
PAIR_DELTA = [1, PW - 2, 1, 1, -1]  # -1 = dummy partner (zero weights)


def _build_program():
    import concourse.bass as bass
    import concourse.tile as tile
    import concourse.mybir as mybir
    from concourse import bacc

    dt = mybir.dt
    f32, f32r, bf16 = dt.float32, dt.float32r, dt.bfloat16
    f8 = dt.float8e4
    u32 = dt.uint32
    AF = mybir.ActivationFunctionType
    ALU = mybir.AluOpType
    AX = mybir.AxisListType
    PM = mybir.MatmulPerfMode
    VP = bass_rust.VecI64Pair

    nc = bacc.Bacc("TRN2", num_devices=8)

    x1h = nc.dram_tensor("x1h", [C, HW], bf16, kind="ExternalInput").ap()
    x1t = nc.dram_tensor("x1t", [C, HW], bf16, kind="ExternalInput").ap()
    x2q = nc.dram_tensor("x2q", [C, HW], f8, kind="ExternalInput").ap()
    wc1 = nc.dram_tensor("wc1", [C, 10, C], f32r, kind="ExternalInput").ap()
    wc2 = nc.dram_tensor("wc2", [C, 10, C], f8, kind="ExternalInput").ap()
    scl = nc.dram_tensor("scl", [C, 4], f32, kind="ExternalInput").ap()
    bmask = nc.dram_tensor("bmask", [C, C], f32, kind="ExternalInput").ap()
    idf = nc.dram_tensor("idf", [C, C], f32, kind="ExternalInput").ap()
    out = nc.dram_tensor("out", [C, HW], bf16, kind="ExternalOutput").ap()

    with tile.TileContext(nc) as tc:
        with (
            tc.tile_pool(name="consts", bufs=1) as consts,
            tc.tile_pool(name="pads", bufs=1) as pads,
            tc.tile_pool(name="attn", bufs=1) as attn,
            tc.tile_pool(name="streams", bufs=3) as streams,
            tc.tile_pool(name="bands", bufs=3) as bands,
        ):
            # ---- constants to SBUF (small ones first on the sync queue) ----
            idfs = consts.tile([C, C], f32)
            nc.sync.dma_start(out=idfs, in_=idf)
            scls = consts.tile([C, 4], f32)
            nc.sync.dma_start(out=scls, in_=scl)
            bmasks = consts.tile([C, C], f32)
            nc.sync.dma_start(out=bmasks, in_=bmask)
            w1s = consts.tile([C, 10, C], f32r)
            nc.gpsimd.dma_start(out=w1s, in_=wc1)
            w2s = consts.tile([C, 10, C], f8)
            nc.gpsimd.dma_start(out=w2s, in_=wc2)
            b1_ap = scls[:, 0:1]   # S_H * conv1_b
            b2_ap = scls[:, 1:2]   # b2' (dw_b + conv2_b * sum(dw))
            bsc_ap = scls[:, 2:3]  # sc_b
            temp_ap = scls[:, 3:4]

            # persistent x1 (bf16) for the residual adds
            x1hs = consts.tile([C, HW], bf16)

            # ---- persistent padded fp8 buffers (flat + 3d views) ----
            p2xF = pads.tile([C, (H + 2) * PW], f8)  # x2*16, zero-pad
            phF = pads.tile([C, (H + 2) * PW], f8)   # h1*4, reflect-pad
            p2x = p2xF.rearrange("p (a b) -> p a b", a=H + 2)
            ph = phF.rearrange("p (a b) -> p a b", a=H + 2)
            # zero the whole x2 pad buffer once (borders stay 0)
            nc.gpsimd.memset(p2xF.bitcast(u32), 0)

            def conv_rhs(tileF, y0, p):
                """[C, 2, 4, 128] overlapping tap-pair window AP."""
                dy, dx = TAPS[2 * p]
                off = (y0 + dy) * PW + dx
                ap = tileF[:, off : off + 1].copy()
                ap.ap = VP([
                    ((H + 2) * PW, C), (PAIR_DELTA[p], 2), (PW, 4), (1, W),
                ])
                return ap

            def sc_rhs(tileF, y0):
                """[C, 2, 4, 128] stride-0 pair dim (hi/lo share the rhs)."""
                off = (1 + y0) * PW + 1
                ap = tileF[:, off : off + 1].copy()
                ap.ap = VP([((H + 2) * PW, C), (0, 2), (PW, 4), (1, W)])
                return ap

            # ================= phase A: Gram + softmax =================
            with (
                tc.tile_pool(name="psG", bufs=1, space="PSUM") as psG,
            ):
                gps = psG.tile([C, C], f32)
                for j in range(8):
                    x1tt = streams.tile([C, 2048], bf16, tag="x1tt", name="x1tt")
                    nc.sync.dma_start(out=x1tt, in_=x1t[:, 2048 * j : 2048 * (j + 1)])
                    for g in range(16):
                        k = 16 * j + g
                        ch = x1tt[:, 128 * g : 128 * (g + 1)]
                        nc.tensor.matmul(
                            out=gps, lhsT=ch, rhs=ch,
                            start=(k == 0), stop=(k == 127),
                            skip_group_check=True,
                        )

                # stream x2 (fp8) into the padded buffer; x1h for residuals
                for j in range(8):
                    nc.sync.dma_start(
                        out=p2x[:, 1 + 16 * j : 17 + 16 * j, 1 : 1 + W],
                        in_=x2q[:, j * 2048 : (j + 1) * 2048],
                    )
                    nc.sync.dma_start(
                        out=x1hs[:, j * 2048 : (j + 1) * 2048],
                        in_=x1h[:, j * 2048 : (j + 1) * 2048],
                    )

                # diag -> norms -> rn
                gi = attn.tile([C, C], f32)
                nc.vector.tensor_mul(out=gi, in0=gps, in1=idfs)
                diag = attn.tile([C, 1], f32)
                nc.vector.reduce_sum(out=diag, in_=gi, axis=AX.X)
                norm = attn.tile([C, 1], f32)
                nc.scalar.activation(out=norm, in_=diag, func=AF.Sqrt)
                nc.vector.tensor_scalar_max(out=norm, in0=norm, scalar1=EPS)
                rn = attn.tile([C, 1], f32)
                nc.vector.reciprocal(out=rn, in_=norm)

                # S = diag(rn) G diag(rn); E = exp(S*temp) * blockmask
                s1 = attn.tile([C, C], f32)
                nc.vector.tensor_scalar_mul(out=s1, in0=gps, scalar1=rn)
                with tc.tile_pool(name="psS", bufs=1, space="PSUM") as psS:
                    s1t = psS.tile([C, C], f32)
                    nc.tensor.transpose(out=s1t, in_=s1, identity=idfs)
                    s2 = attn.tile([C, C], f32)
                    nc.vector.tensor_scalar_mul(out=s2, in0=s1t, scalar1=rn)
                e0 = attn.tile([C, C], f32)
                nc.scalar.activation(out=e0, in_=s2, func=AF.Exp, scale=temp_ap)
                em = attn.tile([C, C], f32)
                nc.vector.tensor_mul(out=em, in0=e0, in1=bmasks)
                rs = attn.tile([C, 1], f32)
                nc.vector.reduce_sum(out=rs, in_=em, axis=AX.X)
                rinv = attn.tile([C, 1], f32)
                nc.vector.reciprocal(out=rinv, in_=rs)
                rinv64 = attn.tile([C, 1], f32)
                nc.vector.tensor_scalar_mul(out=rinv64, in0=rinv, scalar1=S_L)
                # em64[d, c] = E[d, c] * rinv[d] * 64 (E symmetric)
                em64 = attn.tile([C, C], f32r)
                nc.vector.tensor_scalar_mul(out=em64, in0=em, scalar1=rinv64)

                # fused fp8 weights: ltall[:, t, :] = e4m3(64 * E rinv w1_t),
                # slot 9 zero; lsc = hi/lo split of the shortcut column.
                ltall = attn.tile([C, 10, C], f8)
                nc.vector.memset(ltall[:, 9, :].bitcast(u32), 0)
                lsc = attn.tile([C, 2, C], f8)
                with tc.tile_pool(name="psW", bufs=2, space="PSUM") as psW:
                    for p in range(5):
                        lps = psW.tile([C, 2, C], f32, name=f"lps{p}", tag="lps")
                        nc.tensor.matmul(
                            out=lps, lhsT=em64, rhs=w1s[:, 2 * p : 2 * p + 2, :],
                            start=True, stop=True,
                        )
                        if p < 4:
                            nc.scalar.activation(
                                out=ltall[:, 2 * p : 2 * p + 2, :], in_=lps,
                                func=AF.Copy,
                            )
                        else:
                            nc.scalar.activation(
                                out=ltall[:, 8:9, :], in_=lps[:, 0:1, :],
                                func=AF.Copy,
                            )
                            nc.scalar.activation(
                                out=lsc[:, 0:1, :], in_=lps[:, 1:2, :],
                                func=AF.Copy,
                            )
                            # lo = lps - hi  (second fp8 k-tile)
                            nc.vector.scalar_tensor_tensor(
                                out=lsc[:, 1, :], in0=lsc[:, 0, :],
                                scalar=-1.0, in1=lps[:, 1, :],
                                op0=ALU.mult, op1=ALU.add,
                            )

            # ============ phases C/D interleaved per 4-row band ============
            with (
                tc.tile_pool(name="psC", bufs=3, space="PSUM") as psC,
                tc.tile_pool(name="psD", bufs=3, space="PSUM") as psD,
                tc.tile_pool(name="psS2", bufs=2, space="PSUM") as psS2,
                tc.tile_pool(name="x1st", bufs=2) as x1st,
            ):
                state = {}

                def emit_C(b):
                    y0 = 4 * b
                    cps = psC.tile([C, 4, C], f32)
                    for p in range(5):
                        nc.tensor.matmul(
                            out=cps,
                            lhsT=ltall[:, 2 * p : 2 * p + 2, :],
                            rhs=conv_rhs(p2xF, y0, p),
                            start=(p == 0), stop=(p == 4),
                            perf_mode=PM.DoubleRow,
                        )
                    # h1*S_H as fp8 into the reflect-padded buffer
                    nc.scalar.activation(
                        out=ph[:, 1 + y0 : 5 + y0, 1 : 1 + W],
                        in_=cps, func=AF.Prelu,
                        bias=b1_ap, scale=S_H * PS_INV, alpha=SLOPE,
                    )
                    # incremental reflect pad of left/right columns
                    nc.gpsimd.tensor_copy(
                        out=ph[:, 1 + y0 : 5 + y0, 0:1],
                        in_=ph[:, 1 + y0 : 5 + y0, 2:3],
                    )
                    nc.gpsimd.tensor_copy(
                        out=ph[:, 1 + y0 : 5 + y0, PW - 1 : PW],
                        in_=ph[:, 1 + y0 : 5 + y0, PW - 3 : PW - 2],
                    )
                    if b == 0:
                        nc.gpsimd.tensor_copy(out=ph[:, 0:1, :], in_=ph[:, 2:3, :])
                    if b == NB - 1:
                        nc.gpsimd.tensor_copy(
                            out=ph[:, H + 1 : H + 2, :], in_=ph[:, H - 1 : H, :]
                        )

                def emit_D(b):
                    y0 = 4 * b
                    if b % 2 == 0:
                        state["otile"] = x1st.tile(
                            [C, 1024], bf16, tag="otile", name="otile"
                        )
                    otile = state["otile"]
                    x1b = x1hs[:, b * 512 : (b + 1) * 512].rearrange(
                        "p (a b) -> p a b", a=4
                    )
                    dps = psD.tile([C, 4, C], f32)
                    for p in range(5):
                        nc.tensor.matmul(
                            out=dps,
                            lhsT=w2s[:, 2 * p : 2 * p + 2, :],
                            rhs=conv_rhs(phF, y0, p),
                            start=(p == 0), stop=(p == 4),
                            perf_mode=PM.DoubleRow,
                        )
                    sps = psS2.tile([C, 4, C], f32)
                    nc.tensor.matmul(
                        out=sps, lhsT=lsc, rhs=sc_rhs(p2xF, y0),
                        start=True, stop=True, perf_mode=PM.DoubleRow,
                    )
                    # h3 = lrelu(dps/1024 + b2') via max(x, 0.2x) on DVE
                    t1 = bands.tile([C, 4, C], f32, tag="t1")
                    nc.vector.tensor_scalar(
                        out=t1, in0=dps, scalar1=PS_INV, scalar2=b2_ap,
                        op0=ALU.mult, op1=ALU.add,
                    )
                    h3 = bands.tile([C, 4, C], f32, tag="h3")
                    nc.vector.scalar_tensor_tensor(
                        out=h3, in0=t1, scalar=SLOPE, in1=t1,
                        op0=ALU.mult, op1=ALU.max,
                    )
                    # u = sps/1024 (+ sc_b) + x1
                    u = bands.tile([C, 4, C], f32, tag="u")
                    nc.vector.scalar_tensor_tensor(
                        out=u, in0=sps, scalar=PS_INV, in1=x1b,
                        op0=ALU.mult, op1=ALU.add,
                    )
                    ob = otile[:, (b % 2) * 512 : (b % 2 + 1) * 512].rearrange(
                        "p (a b) -> p a b", a=4
                    )
                    nc.gpsimd.tensor_add(out=ob, in0=u, in1=h3)
                    if b % 2 == 1:
                        nc.sync.dma_start(
                            out=out[:, (y0 - 4) * W : (y0 + 4) * W], in_=otile
                        )

                for k in range(NB + 1):
                    if k < NB:
                        emit_C(k)
                    if k >= 1:
                        emit_D(k - 1)

    nc.compile()
    return nc


def _prep_consts(temperature, conv1_w, conv1_b, conv2_w, conv2_b,
                 dw_w, dw_b, sc_w, sc_b):
    f32 = np.float32
    conv1_w = np.asarray(conv1_w, f32)
    conv2_w = np.asarray(conv2_w, f32)
    dw_w = np.asarray(dw_w, f32)
    sc_w = np.asarray(sc_w, f32)
    # conv1 taps as lhsT: wc1[ci, t, co] = conv1_w[co, ci, dy, dx];
    # slot 9 = shortcut 1x1 weights (sc_w transposed)
    wc1 = np.zeros((C, 10, C), f32)
    wc1[:, :9, :] = conv1_w.transpose(1, 2, 3, 0).reshape(C, 9, C)
    wc1[:, 9, :] = sc_w[:, :, 0, 0].T
    # fused conv2+dw taps: wc2[ci, t, co] = conv2_w[co, ci] * dw_w[co, t]
    A2 = conv2_w[:, :, 0, 0]
    Dw = dw_w[:, 0, :, :].reshape(C, 9)
    wc2 = np.zeros((C, 10, C), f32)
    wc2[:, :9, :] = np.einsum("oc,ot->cto", A2, Dw)
    wc2q = np.ascontiguousarray((wc2 * S_W2).astype(E4))
    b2p = np.asarray(dw_b, f32) + np.asarray(conv2_b, f32) * Dw.sum(axis=1)
    temp_b = np.repeat(np.asarray(temperature, f32).reshape(HEADS), HEAD_C)
    scl = np.ascontiguousarray(
        np.stack(
            [S_H * np.asarray(conv1_b, f32), b2p, np.asarray(sc_b, f32), temp_b],
            axis=1,
        )
    )
    bmask = np.kron(np.eye(HEADS, dtype=f32), np.ones((HEAD_C, HEAD_C), f32))
    idf = np.eye(C, dtype=f32)
    return dict(
        wc1=np.ascontiguousarray(wc1), wc2=wc2q, scl=scl,
        bmask=np.ascontiguousarray(bmask), idf=np.ascontiguousarray(idf),
    )


def kernel(
    x1, x2, temperature, conv1_w, conv1_b, conv2_w, conv2_b, dw_w, dw_b, sc_w, sc_b
):
    from concourse.bass_utils import run_bass_kernel_spmd

    if "nc" not in _cache:
        _cache["nc"] = _build_program()
    nc = _cache["nc"]

    x1 = np.ascontiguousarray(np.asarray(x1, np.float32))
    x2 = np.ascontiguousarray(np.asarray(x2, np.float32))
    consts = _prep_consts(
        temperature, conv1_w, conv1_b, conv2_w, conv2_b, dw_w, dw_b, sc_w, sc_b
    )
    in_maps = []
    for b in range(B):
        m = dict(consts)
        x1b = x1[b].reshape(C, HW)
        m["x1h"] = np.ascontiguousarray(x1b.astype(BF16))
        # x1t[p, 128k + c] = x1[c, 128k + p]
        m["x1t"] = np.ascontiguousarray(
            x1b.reshape(C, 128, 128).transpose(2, 1, 0).reshape(C, HW).astype(BF16)
        )
        m["x2q"] = np.ascontiguousarray(
            (x2[b].reshape(C, HW) * S_X2).astype(E4)
        )
        in_maps.append(m)

    res = run_bass_kernel_spmd(nc, in_maps, core_ids=list(range(B)))
    outs = [
        res.results[b]["out"].astype(np.float32).reshape(C, H, W) for b in range(B)
    ]
    return np.stack(outs, axis=0)


# revision 5
# speedup vs baseline: 2.0314x; 1.1941x over previous
"""Trainium2 Bass kernel for nn_Cross_Attention_18425409700231.

Per-sample channel attention (16 heads x 8 channels, L2-normalized over
spatial, softmax over in-head channels) followed by a conv block
(3x3 conv -> LeakyReLU -> 1x1 conv -> reflect-pad depthwise 3x3 ->
LeakyReLU, plus 1x1 shortcut) and a residual add.

Sharding: data-parallel over batch B=8 -> one sample per NeuronCore.

Device algorithm per core (sample b), layouts [C=128 partitions, H*W]:
  A. Gram G = x1 @ x1^T from a host-pretransposed bf16 copy of x1
     (x1t[p, 128k+c] = x1[c, 128k+p]) -> 128 plain bf16 matmuls, no
     PE transposes and no PSUM->SBUF copies.
  B. diag(G) via accum_out; rn = rsqrt(diag); S = rn*G*rn (one PE
     transpose, the second rn folded into the Exp scale with the
     temperature); E = exp(.) masked to the 16 block-diagonal 8x8 head
     blocks with the row-sum from the same op's accum_out. Fused
     attention+conv weights L_t = E diag(rinv*64) w1_t quantized to
     fp8-e4m3; the shortcut column is split hi/lo into two fp8 k-tiles
     for ~bf16 accuracy.
  C. conv1 3x3 over fp8 x2 (host-quantized at scale 16, host-padded to
     the 130x130 layout so the DMA moves full-partition runs) as 5 fp8
     DoubleRow matmuls per 4-row band (taps paired into 2x128 k-tiles
     via overlapping access patterns; DoubleRow runs 0.5 cycles/row =
     4x the f32r rate). Two bands share a 2-bank PSUM tile so one
     8-row ACT Prelu (bias+rescale fused) writes the fp8 (scale 4)
     reflect-padded buffer.
  D. conv2(1x1)+depthwise 3x3 fused into 9 dense taps (reflect pad
     commutes with 1x1 conv), 5 DoubleRow matmuls per band, plus one
     DoubleRow shortcut matmul (hi/lo k-tiles, stride-0 pair dim).
     Assembly: ACT Prelu (bf16) for h3, DVE makes u = sc + x1 (bf16),
     one 2x-mode bf16 DVE add per band-pair, bf16 store, f32 upcast on
     the host.
  C/D are emitted interleaved per band-pair so the PE never drains.
"""

import numpy as np
import ml_dtypes
import bass_rust

B, C, H, W = 8, 128, 128, 128
HW = H * W
HEADS, HEAD_C = 16, 8
SLOPE = 0.2
PW = W + 2   # padded width
PH = H + 2   # padded height
NP = H // 8  # 16 band-pairs (8 rows each)

# fp8 scaling factors
S_X2 = 16.0   # x2 quantized as e4m3(16*x2)
S_L = 64.0    # fused conv1/shortcut weights
S_H = 4.0     # h1 (conv1 output) fp8 scale
S_W2 = 256.0  # fused conv2+dw weights
# conv1 PSUM carries S_L*S_X2 = 1024x; conv2 PSUM carries S_W2*S_H = 1024x;
# shortcut PSUM carries S_L*S_X2 = 1024x.
PS_INV = 1.0 / 1024.0

E4 = ml_dtypes.float8_e4m3
BF16 = ml_dtypes.bfloat16

_cache = {}

# conv taps row-major; pairs (2p, 2p+1), 5th pair = (tap8, zero dummy).
# delta = element offset from tap 2p to tap 2p+1 in the [*, 130, 130] buffer.
TAPS = [(dy, dx) for dy in range(3) for dx in range(3)]
PAIR_DELTA = [1, PW - 2, 1, 1, -1]


def _build_program():
    import concourse.bass as bass
    import concourse.tile as tile
    import concourse.mybir as mybir
    from concourse import bacc

    dt = mybir.dt
    f32, f32r, bf16 = dt.float32, dt.float32r, dt.bfloat16
    f8 = dt.float8e4
    u32 = dt.uint32
    AF = mybir.ActivationFunctionType
    ALU = mybir.AluOpType
    PM = mybir.MatmulPerfMode
    VP = bass_rust.VecI64Pair

    nc = bacc.Bacc("TRN2", num_devices=8)

    x1h = nc.dram_tensor("x1h", [C, HW], bf16, kind="ExternalInput").ap()
    x1t = nc.dram_tensor("x1t", [C, HW], bf16, kind="ExternalInput").ap()
    x2p = nc.dram_tensor("x2p", [C, PH * PW], f8, kind="ExternalInput").ap()
    wc1 = nc.dram_tensor("wc1", [C, 10, C], f32r, kind="ExternalInput").ap()
    wc2 = nc.dram_tensor("wc2", [C, 10, C], f8, kind="ExternalInput").ap()
    scl = nc.dram_tensor("scl", [C, 4], f32, kind="ExternalInput").ap()
    bmask = nc.dram_tensor("bmask", [C, C], f32, kind="ExternalInput").ap()
    idf = nc.dram_tensor("idf", [C, C], f32, kind="ExternalInput").ap()
    out = nc.dram_tensor("out", [C, HW], bf16, kind="ExternalOutput").ap()

    with tile.TileContext(nc) as tc:
        with (
            tc.tile_pool(name="consts", bufs=1) as consts,
            tc.tile_pool(name="pads", bufs=1) as pads,
            tc.tile_pool(name="attn", bufs=1) as attn,
            tc.tile_pool(name="streams", bufs=3) as streams,
            tc.tile_pool(name="bands", bufs=3) as bands,
        ):
            # ---- small consts on the sync queue, big ones on gpsimd ----
            idfs = consts.tile([C, C], f32)
            nc.sync.dma_start(out=idfs, in_=idf)
            scls = consts.tile([C, 4], f32)
            nc.sync.dma_start(out=scls, in_=scl)
            bmasks = consts.tile([C, C], f32)
            nc.sync.dma_start(out=bmasks, in_=bmask)
            w1s = consts.tile([C, 10, C], f32r)
            nc.gpsimd.dma_start(out=w1s, in_=wc1)
            w2s = consts.tile([C, 10, C], f8)
            nc.gpsimd.dma_start(out=w2s, in_=wc2)
            b1_ap = scls[:, 0:1]   # S_H * conv1_b
            b2_ap = scls[:, 1:2]   # b2' = dw_b + conv2_b * sum(dw)
            bsc_ap = scls[:, 2:3]  # sc_b (folded on host when zero)
            temp_ap = scls[:, 3:4]

            # persistent x1 (bf16) for the residual adds
            x1hs = consts.tile([C, HW], bf16)

            # ---- persistent padded fp8 buffers (flat + 3d views) ----
            p2xF = pads.tile([C, PH * PW], f8)  # 16*x2, zero-pad (host-padded)
            phF = pads.tile([C, PH * PW], f8)   # 4*h1, reflect-pad
            ph = phF.rearrange("p (a b) -> p a b", a=PH)

            def conv_rhs(tileF, y0, p):
                """[C, 2, 4, 128] overlapping tap-pair window AP."""
                dy, dx = TAPS[2 * p]
                off = (y0 + dy) * PW + dx
                ap = tileF[:, off : off + 1].copy()
                ap.ap = VP([(PH * PW, C), (PAIR_DELTA[p], 2), (PW, 4), (1, W)])
                return ap

            def sc_rhs(tileF, y0):
                """[C, 2, 4, 128] stride-0 pair dim (hi/lo share the rhs)."""
                off = (1 + y0) * PW + 1
                ap = tileF[:, off : off + 1].copy()
                ap.ap = VP([(PH * PW, C), (0, 2), (PW, 4), (1, W)])
                return ap

            # ================= phase A: Gram + softmax =================
            with (
                tc.tile_pool(name="psG", bufs=1, space="PSUM") as psG,
            ):
                gps = psG.tile([C, C], f32)
                for j in range(8):
                    x1tt = streams.tile([C, 2048], bf16, tag="x1tt", name="x1tt")
                    nc.sync.dma_start(out=x1tt, in_=x1t[:, 2048 * j : 2048 * (j + 1)])
                    for g in range(16):
                        k = 16 * j + g
                        ch = x1tt[:, 128 * g : 128 * (g + 1)]
                        nc.tensor.matmul(
                            out=gps, lhsT=ch, rhs=ch,
                            start=(k == 0), stop=(k == 127),
                            skip_group_check=True,
                        )

                # stream x2 (fp8, host-padded) and x1h (residual)
                for j in range(2):
                    nc.sync.dma_start(
                        out=p2xF[:, j * 8450 : (j + 1) * 8450],
                        in_=x2p[:, j * 8450 : (j + 1) * 8450],
                    )
                for j in range(8):
                    nc.sync.dma_start(
                        out=x1hs[:, j * 2048 : (j + 1) * 2048],
                        in_=x1h[:, j * 2048 : (j + 1) * 2048],
                    )

                # diag -> rn = rsqrt(diag) (norms ~128, eps guard not needed)
                gi = attn.tile([C, C], f32)
                diag = attn.tile([C, 1], f32)
                nc.vector.scalar_tensor_tensor(
                    out=gi, in0=gps, scalar=1.0, in1=idfs,
                    op0=ALU.mult, op1=ALU.mult, accum_out=diag,
                )
                dinv = attn.tile([C, 1], f32)
                nc.vector.reciprocal(out=dinv, in_=diag)
                rn = attn.tile([C, 1], f32)
                nc.scalar.activation(out=rn, in_=dinv, func=AF.Sqrt)
                rn_t = attn.tile([C, 1], f32)
                nc.vector.tensor_mul(out=rn_t, in0=rn, in1=temp_ap)

                # S*temp = (rn G rn)*temp via row scale, transpose, Exp scale
                s1 = attn.tile([C, C], f32)
                nc.vector.tensor_scalar_mul(out=s1, in0=gps, scalar1=rn)
                with tc.tile_pool(name="psS", bufs=1, space="PSUM") as psS:
                    s1t = psS.tile([C, C], f32)
                    nc.tensor.transpose(out=s1t, in_=s1, identity=idfs)
                    e0 = attn.tile([C, C], f32)
                    nc.scalar.activation(out=e0, in_=s1t, func=AF.Exp, scale=rn_t)
                em = attn.tile([C, C], f32)
                rs = attn.tile([C, 1], f32)
                nc.vector.scalar_tensor_tensor(
                    out=em, in0=e0, scalar=1.0, in1=bmasks,
                    op0=ALU.mult, op1=ALU.mult, accum_out=rs,
                )
                rs64 = attn.tile([C, 1], f32)
                nc.vector.tensor_scalar_mul(out=rs64, in0=rs, scalar1=1.0 / S_L)
                rinv64 = attn.tile([C, 1], f32)
                nc.vector.reciprocal(out=rinv64, in_=rs64)
                # em64[d, c] = E[d, c] * rinv[d] * 64 (E symmetric)
                em64 = attn.tile([C, C], f32r)
                nc.vector.tensor_scalar_mul(out=em64, in0=em, scalar1=rinv64)

                # fused fp8 weights: ltall[:, t, :] = e4m3(64 E rinv w1_t),
                # slot 9 zero; lsc = hi/lo split of the shortcut column.
                ltall = attn.tile([C, 10, C], f8)
                nc.vector.memset(ltall[:, 9, :].bitcast(u32), 0)
                lsc = attn.tile([C, 2, C], f8)
                with tc.tile_pool(name="psW", bufs=2, space="PSUM") as psW:
                    for p in range(5):
                        lps = psW.tile([C, 2, C], f32, name=f"lps{p}", tag="lps")
                        nc.tensor.matmul(
                            out=lps, lhsT=em64, rhs=w1s[:, 2 * p : 2 * p + 2, :],
                            start=True, stop=True,
                        )
                        if p < 4:
                            nc.scalar.activation(
                                out=ltall[:, 2 * p : 2 * p + 2, :], in_=lps,
                                func=AF.Copy,
                            )
                        else:
                            nc.scalar.activation(
                                out=ltall[:, 8:9, :], in_=lps[:, 0:1, :],
                                func=AF.Copy,
                            )
                            nc.scalar.activation(
                                out=lsc[:, 0:1, :], in_=lps[:, 1:2, :],
                                func=AF.Copy,
                            )
                            # lo = lps - hi  (second fp8 k-tile)
                            nc.vector.scalar_tensor_tensor(
                                out=lsc[:, 1, :], in0=lsc[:, 0, :],
                                scalar=-1.0, in1=lps[:, 1, :],
                                op0=ALU.mult, op1=ALU.add,
                            )

            # ========= phases C/D interleaved per 8-row band-pair =========
            with (
                tc.tile_pool(name="psC", bufs=2, space="PSUM") as psC,
                tc.tile_pool(name="psD", bufs=3, space="PSUM") as psD,
                tc.tile_pool(name="psS2", bufs=1, space="PSUM") as psS2,
                tc.tile_pool(name="x1st", bufs=2) as x1st,
            ):
                state = {}

                def emit_C(jp):
                    y0 = 8 * jp
                    cps = psC.tile([C, 8, C], f32)
                    for half in range(2):
                        yh = y0 + 4 * half
                        for p in range(5):
                            nc.tensor.matmul(
                                out=cps[:, 4 * half : 4 * half + 4, :],
                                lhsT=ltall[:, 2 * p : 2 * p + 2, :],
                                rhs=conv_rhs(p2xF, yh, p),
                                start=(p == 0), stop=(p == 4),
                                perf_mode=PM.DoubleRow,
                            )
                    # 8 rows of S_H*h1 as fp8 into the reflect-padded buffer
                    nc.scalar.activation(
                        out=ph[:, 1 + y0 : 9 + y0, 1 : 1 + W],
                        in_=cps, func=AF.Prelu,
                        bias=b1_ap, scale=S_H * PS_INV, alpha=SLOPE,
                    )
                    nc.gpsimd.tensor_copy(
                        out=ph[:, 1 + y0 : 9 + y0, 0:1],
                        in_=ph[:, 1 + y0 : 9 + y0, 2:3],
                    )
                    nc.gpsimd.tensor_copy(
                        out=ph[:, 1 + y0 : 9 + y0, PW - 1 : PW],
                        in_=ph[:, 1 + y0 : 9 + y0, PW - 3 : PW - 2],
                    )
                    if jp == 0:
                        nc.gpsimd.tensor_copy(out=ph[:, 0:1, :], in_=ph[:, 2:3, :])
                    if jp == NP - 1:
                        nc.gpsimd.tensor_copy(
                            out=ph[:, H + 1 : H + 2, :], in_=ph[:, H - 1 : H, :]
                        )

                def emit_D(b):
                    y0 = 4 * b
                    half = b % 2
                    if half == 0:
                        state["otile"] = x1st.tile(
                            [C, 1024], bf16, tag="otile", name="otile"
                        )
                        state["h3_8"] = bands.tile(
                            [C, 8, C], bf16, tag="h3", name="h3"
                        )
                        state["u8"] = bands.tile(
                            [C, 8, C], bf16, tag="u8", name="u8"
                        )
                    otile, h3_8, u8 = state["otile"], state["h3_8"], state["u8"]
                    x1b = x1hs[:, b * 512 : (b + 1) * 512].rearrange(
                        "p (a b) -> p a b", a=4
                    )
                    dps = psD.tile([C, 4, C], f32)
                    for p in range(5):
                        nc.tensor.matmul(
                            out=dps,
                            lhsT=w2s[:, 2 * p : 2 * p + 2, :],
                            rhs=conv_rhs(phF, y0, p),
                            start=(p == 0), stop=(p == 4),
                            perf_mode=PM.DoubleRow,
                        )
                    sps = psS2.tile([C, 4, C], f32)
                    nc.tensor.matmul(
                        out=sps, lhsT=lsc, rhs=sc_rhs(p2xF, y0),
                        start=True, stop=True, perf_mode=PM.DoubleRow,
                    )
                    # h3 = lrelu(dps/1024 + b2') as bf16 (ACT)
                    nc.scalar.activation(
                        out=h3_8[:, 4 * half : 4 * half + 4, :],
                        in_=dps, func=AF.Prelu,
                        bias=b2_ap, scale=PS_INV, alpha=SLOPE,
                    )
                    # u = sps/1024 + x1  (sc_b folded into x1h on host)
                    nc.vector.scalar_tensor_tensor(
                        out=u8[:, 4 * half : 4 * half + 4, :],
                        in0=sps, scalar=PS_INV, in1=x1b,
                        op0=ALU.mult, op1=ALU.add,
                    )
                    if half == 1:
                        ob = otile.rearrange("p (a b) -> p a b", a=8)
                        nc.vector.tensor_add(out=ob, in0=u8, in1=h3_8)
                        nc.sync.dma_start(
                            out=out[:, (y0 - 4) * W : (y0 + 4) * W], in_=otile
                        )

                for jp in range(NP + 1):
                    if jp < NP:
                        emit_C(jp)
                    if jp >= 1:
                        emit_D(2 * (jp - 1))
                        emit_D(2 * jp - 1)

    nc.compile()
    return nc


def _prep_consts(temperature, conv1_w, conv1_b, conv2_w, conv2_b,
                 dw_w, dw_b, sc_w, sc_b):
    f32 = np.float32
    conv1_w = np.asarray(conv1_w, f32)
    conv2_w = np.asarray(conv2_w, f32)
    dw_w = np.asarray(dw_w, f32)
    sc_w = np.asarray(sc_w, f32)
    # conv1 taps as lhsT: wc1[ci, t, co] = conv1_w[co, ci, dy, dx];
    # slot 9 = shortcut 1x1 weights (sc_w transposed)
    wc1 = np.zeros((C, 10, C), f32)
    wc1[:, :9, :] = conv1_w.transpose(1, 2, 3, 0).reshape(C, 9, C)
    wc1[:, 9, :] = sc_w[:, :, 0, 0].T
    # fused conv2+dw taps: wc2[ci, t, co] = conv2_w[co, ci] * dw_w[co, t]
    A2 = conv2_w[:, :, 0, 0]
    Dw = dw_w[:, 0, :, :].reshape(C, 9)
    wc2 = np.zeros((C, 10, C), f32)
    wc2[:, :9, :] = np.einsum("oc,ot->cto", A2, Dw)
    wc2q = np.ascontiguousarray((wc2 * S_W2).astype(E4))
    b2p = np.asarray(dw_b, f32) + np.asarray(conv2_b, f32) * Dw.sum(axis=1)
    temp_b = np.repeat(np.asarray(temperature, f32).reshape(HEADS), HEAD_C)
    scl = np.ascontiguousarray(
        np.stack(
            [S_H * np.asarray(conv1_b, f32), b2p, np.asarray(sc_b, f32), temp_b],
            axis=1,
        )
    )
    bmask = np.kron(np.eye(HEADS, dtype=f32), np.ones((HEAD_C, HEAD_C), f32))
    idf = np.eye(C, dtype=f32)
    return dict(
        wc1=np.ascontiguousarray(wc1), wc2=wc2q, scl=scl,
        bmask=np.ascontiguousarray(bmask), idf=np.ascontiguousarray(idf),
    )


def kernel(
    x1, x2, temperature, conv1_w, conv1_b, conv2_w, conv2_b, dw_w, dw_b, sc_w, sc_b
):
    from concourse.bass_utils import run_bass_kernel_spmd

    if "nc" not in _cache:
        _cache["nc"] = _build_program()
    nc = _cache["nc"]

    x1 = np.ascontiguousarray(np.asarray(x1, np.float32))
    x2 = np.ascontiguousarray(np.asarray(x2, np.float32))
    consts = _prep_consts(
        temperature, conv1_w, conv1_b, conv2_w, conv2_b, dw_w, dw_b, sc_w, sc_b
    )
    # device computes out = h3 + sps/1024 + x1h; fold sc_b into x1h
    scb = np.asarray(sc_b, np.float32).reshape(C, 1)
    in_maps = []
    for b in range(B):
        m = dict(consts)
        x1b = x1[b].reshape(C, HW)
        m["x1h"] = np.ascontiguousarray((x1b + scb).astype(BF16))
        # x1t[p, 128k + c] = x1[c, 128k + p]
        m["x1t"] = np.ascontiguousarray(
            x1b.reshape(C, 128, 128).transpose(2, 1, 0).reshape(C, HW).astype(BF16)
        )
        x2pad = np.zeros((C, PH, PW), E4)
        x2pad[:, 1 : 1 + H, 1 : 1 + W] = (
            x2[b].reshape(C, H, W) * S_X2
        ).astype(E4)
        m["x2p"] = np.ascontiguousarray(x2pad.reshape(C, PH * PW))
        in_maps.append(m)

    res = run_bass_kernel_spmd(nc, in_maps, core_ids=list(range(B)))
    outs = [
        res.results[b]["out"].astype(np.float32).reshape(C, H, W) for b in range(B)
    ]
    return np.stack(outs, axis=0)
